# revision 19
# baseline (speedup 1.0000x reference)
"""Trainium2 Bass kernel for AttentionAgger:
out = softmax(mask ? -inf : (Q@WQ.T+bq) @ (K@WK.T+bk).T / sqrt(512)) @ V

Sharding: rows of Q across 8 cores. K's projection is ALSO sharded: each
core projects 1/8 of the keys (1024) and the Kp panels are exchanged via
two DRAM AllGathers (split in halves so the second gather overlaps
attention on the first half's chunks). V and mask stay replicated reads.

All tensors are staged on host in transposed layouts so every matmul
contraction lands on the partition axis with zero on-device transposes of
the big operands. Projections and PV run in bf16; the scores matmul runs
in fp8e4 with DoubleRow perf mode (2 contraction subtiles per instruction
at 2x throughput). Empirical rel_err 0.013 (gate 2e-2).
"""

import math
from contextlib import ExitStack

import numpy as np
import ml_dtypes

import concourse.bacc as bacc
import concourse.tile as tile
from concourse import mybir
from concourse.bass_utils import run_bass_kernel_spmd
from concourse.masks import make_identity

NCORES = 8
NQ, NK, QD, MD, VD = 8192, 8192, 512, 512, 512
NQL = NQ // NCORES            # 1024 query rows per core
NKL = NK // NCORES            # 1024 keys projected per core
SCALE = 1.0 / math.sqrt(MD)
BF16 = mybir.dt.bfloat16
FP8 = mybir.dt.float8e4
F32 = mybir.dt.float32
P = 128
DR = mybir.MatmulPerfMode.DoubleRow


def _build(nql=NQL, nk=NK, repeat=1, count_iters=False, nonce=0):
    """Build + compile the per-core Bass program.

    repeat/count_iters/nonce are for timing builds: repeat hardware-loops the
    whole body; count_iters adds an accumulating iteration counter output;
    nonce adds a dummy input to force a distinct HLO signature (defeats
    stale compile-cache aliasing).
    """
    RP = 512                  # rows per panel
    CK = 512                  # keys per chunk
    npanel = max(1, nql // RP)
    rp = min(RP, nql)
    nchunk = nk // CK
    n_ms = MD // P            # 4 mdim subtiles
    n_qs = QD // P            # 4 qdim subtiles
    n_rs = rp // P            # rows subtiles per panel
    n_ks = CK // P            # 4 key subtiles per chunk
    nhalf = NKL // CK         # 2 local shard halves (one gather each)
    # Gathered half h holds global chunks {2c+h for core c}; process all
    # h=0 chunks first so attention never waits on the second gather.
    chunk_order = [2 * r + h for h in range(nhalf) for r in range(NCORES)]

    nc = bacc.Bacc("TRN2", target_bir_lowering=False, debug=False,
                   num_devices=NCORES)
    qt = nc.dram_tensor("qt", [QD, nql], BF16, kind="ExternalInput")
    kt = nc.dram_tensor("kt", [QD, NKL], BF16, kind="ExternalInput")
    v = nc.dram_tensor("v", [nk, VD], BF16, kind="ExternalInput")
    mt = nc.dram_tensor("mt", [nk, nql], mybir.dt.uint8, kind="ExternalInput")
    wqt = nc.dram_tensor("wqt", [QD, MD], BF16, kind="ExternalInput")
    wkt = nc.dram_tensor("wkt", [QD, MD], BF16, kind="ExternalInput")
    bq = nc.dram_tensor("bq", [MD, 1], F32, kind="ExternalInput")
    bk = nc.dram_tensor("bk", [MD, 1], F32, kind="ExternalInput")
    if nonce:
        nc.dram_tensor("nonce", [1, nonce], F32, kind="ExternalInput")
    out = nc.dram_tensor("out", [nql, VD], F32, kind="ExternalOutput")
    itercnt = (nc.dram_tensor("itercnt", [1, 1], F32, kind="ExternalOutput")
               if count_iters else None)

    with tile.TileContext(nc) as tc:
        with ExitStack() as ctx:
            persist = ctx.enter_context(tc.tile_pool(name="persist", bufs=1))
            kpp = ctx.enter_context(tc.tile_pool(name="kpp", bufs=2))
            psc = ctx.enter_context(tc.tile_pool(name="psc", bufs=4, space="PSUM"))
            ppv = ctx.enter_context(tc.tile_pool(name="ppv", bufs=4, space="PSUM"))
            mpool = ctx.enter_context(tc.tile_pool(name="mpool", bufs=16))
            ptpool = ctx.enter_context(tc.tile_pool(name="ptpool", bufs=10))
            spool = ctx.enter_context(tc.tile_pool(name="spool", bufs=3))
            apool = ctx.enter_context(tc.tile_pool(name="apool", bufs=2))
            opool = ctx.enter_context(tc.tile_pool(name="opool", bufs=2))
            dram = ctx.enter_context(tc.tile_pool(name="dram", bufs=1,
                                                  space="DRAM"))

            def kp_proj_gather(emit_cc):
                """Kp shard projection + (optionally) the two allgathers.

                Core c projects its keys [c*NKL, (c+1)*NKL) as
                Kp.T = WK @ Kshard.T + bk : [MD, NKL] fp8, then the two
                CK-sized halves are allgathered DRAM->DRAM. emit_cc=False
                (timing loop body) re-runs the projection + staging DMAs but
                reuses the pre-loop gather output: this runtime cannot run
                NEFF collectives inside a hardware loop.
                """
                wk_sb = persist.tile([P, n_qs, MD], BF16, tag="wk", name="wk_sb")
                bk_sb = persist.tile([P, n_ms], F32, tag="bk", name="bk_sb")
                kt_sb = persist.tile([P, n_qs, NKL], BF16, tag="kt", name="kt_sb")
                for qs in range(n_qs):
                    nc.sync.dma_start(out=wk_sb[:, qs, :],
                                      in_=wkt[qs * P:(qs + 1) * P, :])
                    nc.sync.dma_start(out=kt_sb[:, qs, :],
                                      in_=kt[qs * P:(qs + 1) * P, :])
                for ms in range(n_ms):
                    nc.sync.dma_start(out=bk_sb[:, ms:ms + 1],
                                      in_=bk[ms * P:(ms + 1) * P, 0:1])
                gouts = []
                for h in range(nhalf):
                    gin = dram.tile([MD, CK], FP8, tag=f"gin{h}",
                                    name=f"gin{h}")
                    gout = dram.tile([NCORES * MD, CK], FP8, tag=f"gout{h}",
                                     name=f"gout{h}")
                    kps = kpp.tile([P, n_ms, CK], FP8, tag="kps", name="kps")
                    for ms in range(n_ms):
                        ps = psc.tile([P, CK], F32, tag="st", name="ps_k")
                        for qs in range(n_qs):
                            nc.tensor.matmul(
                                ps[:],
                                lhsT=wk_sb[:, qs, ms * P:(ms + 1) * P],
                                rhs=kt_sb[:, qs, h * CK:(h + 1) * CK],
                                start=(qs == 0), stop=(qs == n_qs - 1))
                        nc.scalar.activation(
                            out=kps[:, ms, :], in_=ps[:],
                            func=mybir.ActivationFunctionType.Identity,
                            bias=bk_sb[:, ms:ms + 1], scale=1.0)
                    for ms in range(n_ms):
                        nc.sync.dma_start(out=gin[ms * P:(ms + 1) * P, :],
                                          in_=kps[:, ms, :])
                    if emit_cc:
                        nc.gpsimd.collective_compute(
                            "AllGather",
                            mybir.AluOpType.bypass,
                            replica_groups=[list(range(NCORES))],
                            ins=[gin.opt()],
                            outs=[gout.opt()],
                        )
                    gouts.append(gout)
                return gouts

            def body(cc_gouts, emit_cc, _iv=None):
                # ---- constants ----
                wq_sb = persist.tile([P, n_qs, MD], BF16, tag="wq", name="wq_sb")
                bq_sb = persist.tile([P, n_ms], F32, tag="bq", name="bq_sb")
                ident = persist.tile([P, P], F32, tag="ident", name="ident")
                make_identity(nc, ident)
                gouts = kp_proj_gather(emit_cc)
                if cc_gouts is not None:
                    gouts = cc_gouts
                kpt = persist.tile([P, n_ms, nk], FP8, tag="kpt", name="kpt",
                                   bufs=2)
                qt_sb = persist.tile([P, n_qs, nql], BF16, tag="qt", name="qt_sb")
                for qs in range(n_qs):
                    nc.sync.dma_start(out=wq_sb[:, qs, :],
                                      in_=wqt[qs * P:(qs + 1) * P, :])
                    nc.sync.dma_start(out=qt_sb[:, qs, :],
                                      in_=qt[qs * P:(qs + 1) * P, :])
                for ms in range(n_ms):
                    nc.sync.dma_start(out=bq_sb[:, ms:ms + 1],
                                      in_=bq[ms * P:(ms + 1) * P, 0:1])

                # ---- Qp.T = WQ @ Q.T + bq : [MD, nql] fp8, resident ----
                # (overlaps the gathers.) fp8e4 storage feeds the DoubleRow
                # scores matmul. Empirical rel_err 0.013 vs 0.0027 all-bf16.
                qpt = persist.tile([P, n_ms, nql], FP8, tag="qpt", name="qpt",
                                   bufs=2)
                rcw = min(CK, nql)
                for rc in range(nql // rcw):
                    for ms in range(n_ms):
                        ps = psc.tile([P, rcw], F32, tag="st", name="ps_q")
                        for qs in range(n_qs):
                            nc.tensor.matmul(
                                ps[:],
                                lhsT=wq_sb[:, qs, ms * P:(ms + 1) * P],
                                rhs=qt_sb[:, qs, rc * rcw:(rc + 1) * rcw],
                                start=(qs == 0), stop=(qs == n_qs - 1))
                        nc.scalar.activation(
                            out=qpt[:, ms, rc * rcw:(rc + 1) * rcw], in_=ps[:],
                            func=mybir.ActivationFunctionType.Identity,
                            bias=bq_sb[:, ms:ms + 1], scale=1.0)

                # ---- readback gathered Kp into SBUF ----
                # gout[h] rows [r*MD, (r+1)*MD) hold core r's half-h panel,
                # i.e. global chunk 2r+h.
                for h in range(nhalf):
                    for r in range(NCORES):
                        kc = 2 * r + h
                        for ms in range(n_ms):
                            nc.sync.dma_start(
                                out=kpt[:, ms, kc * CK:(kc + 1) * CK],
                                in_=gouts[h][r * MD + ms * P:
                                             r * MD + (ms + 1) * P, :])

                v_sb = persist.tile([P, nchunk * n_ks, VD], BF16, tag="v",
                                    name="v_sb")

                def load_v_chunk(kc):
                    for ks in range(n_ks):
                        nc.sync.dma_start(
                            out=v_sb[:, kc * n_ks + ks, :],
                            in_=v[kc * CK + ks * P: kc * CK + (ks + 1) * P, :])

                def attn_chunk(pn, kc, sumacc, pv):
                    r0 = pn * rp
                    # prefetch the chunk's mask tiles so the DMAs overlap the
                    # score matmuls instead of sitting on the exp->mul path
                    mtts = []
                    for ks in range(n_ks):
                        mtt = mpool.tile([P, rp], mybir.dt.uint8, tag="mt", name="mtt")
                        nc.sync.dma_start(
                            out=mtt[:],
                            in_=mt[kc * CK + ks * P: kc * CK + (ks + 1) * P,
                                   r0:r0 + rp])
                        mtts.append(mtt)
                    pts = []
                    for ks in range(n_ks):
                        st = psc.tile([P, rp], F32, tag="st", name="st")
                        for mp in range(n_ms // 2):
                            nc.tensor.matmul(
                                st[:],
                                lhsT=kpt[:, 2 * mp:2 * mp + 2,
                                         kc * CK + ks * P: kc * CK + (ks + 1) * P],
                                rhs=qpt[:, 2 * mp:2 * mp + 2, r0:r0 + rp],
                                start=(mp == 0), stop=(mp == n_ms // 2 - 1),
                                perf_mode=DR)
                        pt = ptpool.tile([P, rp], BF16, tag="pt", name="pt")
                        nc.scalar.activation(
                            out=pt[:], in_=st[:],
                            func=mybir.ActivationFunctionType.Exp,
                            scale=SCALE)
                        nc.vector.tensor_mul(pt[:], pt[:], mtts[ks][:])
                        pts.append(pt)
                    t01 = spool.tile([P, rp], BF16, tag="t01", name="t01")
                    t23 = spool.tile([P, rp], BF16, tag="t23", name="t23")
                    nc.vector.tensor_add(t01[:], pts[0][:], pts[1][:])
                    nc.vector.tensor_add(t23[:], pts[2][:], pts[3][:])
                    tsum = spool.tile([P, rp], BF16, tag="tsum", name="tsum")
                    nc.vector.tensor_add(tsum[:], t01[:], t23[:])
                    nc.vector.tensor_add(sumacc[:], sumacc[:], tsum[:])
                    first = (kc == chunk_order[0])
                    last = (kc == chunk_order[-1])
                    for ks in range(n_ks):
                        for rs in range(n_rs):
                            nc.tensor.matmul(
                                pv[rs][:],
                                lhsT=pts[ks][:, rs * P:(rs + 1) * P],
                                rhs=v_sb[:, kc * n_ks + ks, :],
                                start=(first and ks == 0),
                                stop=(last and ks == n_ks - 1))

                def finalize_panel(pn, sumacc, pv):
                    r0 = pn * rp
                    for rs in range(n_rs):
                        tr = psc.tile([P, P], F32, tag="st", name="tr")
                        nc.tensor.transpose(tr[:],
                                            sumacc[:, rs * P:(rs + 1) * P],
                                            ident[:])
                        sums = spool.tile([P, 1], F32, tag="sums", name="sums")
                        nc.vector.reduce_sum(sums[:], tr[:],
                                             axis=mybir.AxisListType.X)
                        rsum = spool.tile([P, 1], F32, tag="rsum", name="rsum")
                        nc.vector.reciprocal(rsum[:], sums[:])
                        ot = opool.tile([P, VD], F32, tag="ot", name="ot")
                        nc.scalar.mul(ot[:], pv[rs][:], rsum[:])
                        nc.sync.dma_start(
                            out=out[r0 + rs * P: r0 + (rs + 1) * P, :],
                            in_=ot[:])

                # panel 0 fused with the V loads over key chunks
                sumacc0 = apool.tile([P, rp], F32, tag="sumacc", name="sumacc")
                nc.vector.memset(sumacc0[:], 0.0)
                pv0 = [ppv.tile([P, VD], F32, tag="pv", name=f"pv{rs}")
                       for rs in range(n_rs)]
                for kc in chunk_order:
                    load_v_chunk(kc)
                    attn_chunk(0, kc, sumacc0, pv0)
                finalize_panel(0, sumacc0, pv0)

                # remaining panels: kpt + v fully resident
                for pn in range(1, npanel):
                    sumacc = apool.tile([P, rp], F32, tag="sumacc",
                                        name="sumacc")
                    nc.vector.memset(sumacc[:], 0.0)
                    pv = [ppv.tile([P, VD], F32, tag="pv", name=f"pv{rs}")
                          for rs in range(n_rs)]
                    for kc in chunk_order:
                        attn_chunk(pn, kc, sumacc, pv)
                    finalize_panel(pn, sumacc, pv)

                if itercnt is not None:
                    one = spool.tile([1, 1], F32, tag="one", name="one")
                    nc.vector.memset(one[:], 1.0)
                    nc.gpsimd.dma_start(out=itercnt[0:1, 0:1], in_=one[:],
                                        accum_op=mybir.AluOpType.add)

            if repeat == 1:
                body(None, True)
            else:
                # NEFF collectives cannot execute inside a hardware loop on
                # this runtime (mesh desync), so gather once up front; the
                # loop body still pays the shard projection, gin staging and
                # gout readback DMAs every iteration.
                pre_gouts = kp_proj_gather(emit_cc=True)
                engines = (mybir.EngineType.PE, mybir.EngineType.Activation,
                           mybir.EngineType.DVE, mybir.EngineType.SP,
                           mybir.EngineType.Pool)
                with tc.For_i(0, repeat, 1, hint_engines=engines) as _i:
                    body(pre_gouts, False, _i)

    nc.compile()
    return nc


_cache = {}


def _get_nc():
    if "nc" not in _cache:
        _cache["nc"] = _build()
    return _cache["nc"]


def _prep_inputs(Q, K, V, WQ_w, WQ_b, WK_w, WK_b, mask):
    bf16 = ml_dtypes.bfloat16
    f32 = np.float32
    Q = np.asarray(Q, dtype=f32)
    K = np.asarray(K, dtype=f32)
    V = np.asarray(V, dtype=f32)
    QT = np.ascontiguousarray(Q.T).astype(bf16)          # [QD, NQ]
    KT = np.ascontiguousarray(K.T).astype(bf16)          # [QD, NK]
    Vb = np.ascontiguousarray(V).astype(bf16)            # [NK, VD]
    keep = np.logical_not(np.asarray(mask))              # True = keep
    MTb = np.ascontiguousarray(keep.T).astype(np.uint8)  # [NK, NQ] {0,1}
    WQT = np.ascontiguousarray(np.asarray(WQ_w, dtype=f32).T).astype(bf16)
    WKT = np.ascontiguousarray(np.asarray(WK_w, dtype=f32).T).astype(bf16)
    BQ = np.asarray(WQ_b, dtype=f32).reshape(MD, 1).copy()
    BK = np.asarray(WK_b, dtype=f32).reshape(MD, 1).copy()

    in_maps = []
    for c in range(NCORES):
        sl = slice(c * NQL, (c + 1) * NQL)
        kl = slice(c * NKL, (c + 1) * NKL)
        in_maps.append({
            "qt": np.ascontiguousarray(QT[:, sl]),
            "kt": np.ascontiguousarray(KT[:, kl]),
            "v": Vb,
            "mt": np.ascontiguousarray(MTb[:, sl]),
            "wqt": WQT,
            "wkt": WKT,
            "bq": BQ,
            "bk": BK,
        })
    return in_maps


def kernel(Q, K, V, WQ_w, WQ_b, WK_w, WK_b, mask):
    nc = _get_nc()
    in_maps = _prep_inputs(Q, K, V, WQ_w, WQ_b, WK_w, WK_b, mask)
    res = run_bass_kernel_spmd(nc, in_maps, core_ids=list(range(NCORES)))
    return np.concatenate([res.results[c]["out"] for c in range(NCORES)],
                          axis=0)
